# revision 4
# baseline (speedup 1.0000x reference)
"""Trainium2 Bass kernel for the DataReloadingQNN problem.

Math: layers 0..4 and the shared layer-5 gates B_q = RZ RY RZ are
sample-independent -> shared state v5.  The per-sample part is
    state_b = P . prod_q RY_q(x_bq) . v5          (P = CNOT chain)
RY_q = c_q I + s_q J_q.  Expanding qubits 2..10 gives
    t_b = sum_{m<512} W[b,m] u_m                  (matmul, K=512)
with u_m = J^m v5 and the CNOT permutation P folded into columns.
Qubits 0 and 1 are applied after the matmul as per-sample butterflies:
P is GF(2)-linear, so in P-space the qubit-1 pairing is a ^ 1023
(reversal within each half) and the qubit-0 pairing is a ^ 2047 (full
reversal) -- both are full-width contiguous ops with a reversed (stride
-1) second operand, with uniform sign per 512-column chunk:
    q1: sign [-,+,+,-] over chunks 0..3, pairs (0,1),(2,3)
    q0: sign [-,-,+,+],                  pairs (0,3),(1,2)
U columns are stored de-interleaved: cols 0..2047 = re, 2048..4095 = im
(butterfly coefficients are real, so planes never mix).
Device work per core (1024 samples):
  1. cos/sin of x/2 on ScalarE
  2. W (128 x 512 per sample-tile) by doubling on VectorE; PE transpose
  3. t = W @ U as bf16 matmuls, K=512, N=4096 (PSUM, 4 banks/plane)
  4. butterflies: ScalarE muls (PSUM), VectorE scalar_tensor_tensor,
     GpSimd muls (SBUF), DMA out f32
Inputs are sharded batch-wise across 8 cores; U (params-derived)
replicated.  Host reassembles (B, 2048, 2) from the two planes.
"""
import numpy as np
import ml_dtypes

import concourse.bass as bass
import concourse.bacc as bacc
import concourse.tile as tile
from concourse import mybir
from concourse.bass_utils import run_bass_kernel_spmd

N = 11
DIM = 2048
BATCH = 8192
NCORES = 8
BSH = BATCH // NCORES          # 1024 samples per core
NTILES = BSH // 128            # 8 sample-tiles per core
NPULL = 2                      # qubits 0,1 pulled out of the expansion
K = 512                        # contraction dim (qubits 2..10 expanded)
KT = K // 128                  # 4 k-tiles
NW = 512                       # columns per chunk (4 chunks per plane)
F32 = mybir.dt.float32
BF16 = mybir.dt.bfloat16

# ---------------------------------------------------------------- host math

def _rz(phi):
    e = np.exp(-0.5j * phi)
    return np.array([[e, 0], [0, np.conj(e)]], dtype=np.complex128)


def _ry(theta):
    t = 0.5 * theta
    c, s = np.cos(t), np.sin(t)
    return np.array([[c, -s], [s, c]], dtype=np.complex128)


def _apply_1q_rows(rows, U, q):
    R = rows.shape[0]
    st = rows.reshape(R, 2 ** q, 2, 2 ** (N - 1 - q))
    st = np.einsum('ab,rxby->rxay', U, st)
    return st.reshape(R, DIM)


def _apply_cnot_rows(rows, c):
    R = rows.shape[0]
    st = rows.reshape(R, 2 ** c, 2, 2, 2 ** (N - 2 - c))
    st = np.stack([st[:, :, 0], st[:, :, 1, ::-1]], axis=2)
    return st.reshape(R, DIM)


def build_u_matrix(params):
    """(6,11,3) f32 -> U (512, 4096) f64: cols 0..2047 re, 2048.. im.

    Qubits 0,1 are NOT expanded (pulled out as device-side butterflies);
    expansion bit i corresponds to qubit i+2."""
    p = params.astype(np.float64)
    v = np.zeros((1, DIM), dtype=np.complex128)
    v[0, 0] = 1.0
    for l in range(5):
        for q in range(N):
            v = _apply_1q_rows(v, _rz(p[l, q, 0]), q)
            v = _apply_1q_rows(v, _ry(p[l, q, 1]), q)
            v = _apply_1q_rows(v, _rz(p[l, q, 2]), q)
        for c in range(N - 1):
            v = _apply_cnot_rows(v, c)
    for q in range(N):
        B = _rz(p[5, q, 2]) @ _ry(p[5, q, 1]) @ _rz(p[5, q, 0])
        v = _apply_1q_rows(v, B, q)

    J = np.array([[0, -1], [1, 0]], dtype=np.complex128)
    rows = v
    for q in range(NPULL, N):      # expand qubits 2..10; bit i <-> qubit i+2
        rc = _apply_1q_rows(rows, J, q)
        rows = np.concatenate([rows, rc], axis=0)

    g = np.arange(DIM)[None, :]
    for c in range(N - 1):
        g = _apply_cnot_rows(g.astype(np.float64), c).astype(np.int64)
    rows = rows[:, g[0]]

    U = np.empty((K, 2 * DIM), dtype=np.float64)
    U[:, 0:DIM] = rows.real
    U[:, DIM:2 * DIM] = rows.imag
    return U

# ------------------------------------------------------------- bass kernel

def build_kernel():
    nc = bacc.Bacc()
    x_d = nc.dram_tensor("x", (BSH, N), F32, kind="ExternalInput")
    u_d = nc.dram_tensor("u", (KT, 128, 2 * DIM), BF16, kind="ExternalInput")
    id_d = nc.dram_tensor("ident", (128, 128), BF16, kind="ExternalInput")
    out_d = nc.dram_tensor("out", (BSH, 2 * DIM), F32, kind="ExternalOutput")

    MULT = mybir.AluOpType.mult
    ADD = mybir.AluOpType.add
    SUB = mybir.AluOpType.subtract

    with tile.TileContext(nc) as tc:
        with (
            tc.tile_pool(name="const", bufs=1) as const_pool,
            tc.tile_pool(name="wbuild", bufs=2) as wbuild_pool,
            tc.tile_pool(name="wt", bufs=1) as wt_pool,
            tc.tile_pool(name="uin", bufs=2) as u_pool,
            tc.tile_pool(name="ys", bufs=2) as y_pool,
            tc.tile_pool(name="outs", bufs=2) as out_pool,
            tc.tile_pool(name="tmps", bufs=2) as tmp_pool,
        ):
            ident = const_pool.tile([128, 128], BF16)
            nc.gpsimd.dma_start(ident[:], id_d[:])

            # x: (1024, 11) -> sbuf (128, 8*11) in one DMA; sample-tile t
            # lives in cols [t*11, (t+1)*11)
            x_sb = const_pool.tile([128, NTILES * N], F32)
            x_r = x_d.rearrange("(t p) f -> p t f", p=128)
            nc.gpsimd.dma_start(x_sb[:].rearrange("p (t f) -> p t f", f=N), x_r)

            cos_sb = const_pool.tile([128, NTILES * N], F32)
            sin_sb = const_pool.tile([128, NTILES * N], F32)
            hp_t = const_pool.tile([128, 1], F32)
            zr_t = const_pool.tile([128, 1], F32)
            nc.vector.memset(hp_t[:], float(np.pi / 2))
            nc.vector.memset(zr_t[:], 0.0)
            # cos(t) = sin(pi/2 - t): keeps Sin args in (-pi/2, pi/2], the
            # ACT table is inaccurate beyond pi
            nc.scalar.activation(cos_sb[:], x_sb[:],
                                 mybir.ActivationFunctionType.Sin,
                                 bias=hp_t[:], scale=-0.5)
            nc.scalar.activation(sin_sb[:], x_sb[:],
                                 mybir.ActivationFunctionType.Sin,
                                 bias=zr_t[:], scale=0.5)

            # Phase A: build transposed W (qubits 2..10) per sample-tile
            wts = []
            with tc.tile_pool(name="ptr", bufs=2,
                              space=bass.MemorySpace.PSUM) as ptr_pool:
                for t in range(NTILES):
                    col = t * N + NPULL   # first expanded qubit
                    wa = wbuild_pool.tile([128, K // 2], F32, tag="wa")
                    wb = wbuild_pool.tile([128, K // 2], F32, tag="wb")
                    nc.vector.tensor_copy(wa[:, 0:1], cos_sb[:, col:col + 1])
                    nc.vector.tensor_copy(wa[:, 1:2], sin_sb[:, col:col + 1])
                    cur, nxt = wa, wb
                    nq = N - NPULL        # 9 expanded qubits
                    for j in range(1, nq - 1):
                        half = 1 << j
                        nc.vector.tensor_scalar_mul(
                            nxt[:, 0:half], cur[:, 0:half],
                            cos_sb[:, col + j:col + j + 1])
                        nc.vector.tensor_scalar_mul(
                            nxt[:, half:2 * half], cur[:, 0:half],
                            sin_sb[:, col + j:col + j + 1])
                        cur, nxt = nxt, cur
                    # final doubling writes bf16 directly
                    j = nq - 1
                    half = 1 << j
                    wbf = wbuild_pool.tile([128, K], BF16, tag="wbf")
                    nc.vector.tensor_scalar_mul(
                        wbf[:, 0:half], cur[:, 0:half],
                        cos_sb[:, col + j:col + j + 1])
                    nc.vector.tensor_scalar_mul(
                        wbf[:, half:2 * half], cur[:, 0:half],
                        sin_sb[:, col + j:col + j + 1])

                    wt = wt_pool.tile([128, KT * 128], BF16, tag=f"wt{t}")
                    for k in range(KT):
                        ptr = ptr_pool.tile([128, 128], BF16)
                        nc.tensor.transpose(
                            ptr[:], wbf[:, k * 128:(k + 1) * 128], ident[:])
                        nc.vector.tensor_copy(
                            wt[:, k * 128:(k + 1) * 128], ptr[:])
                    wts.append(wt)

            # Phase B: per plane (re, im): load 4 U chunks, then per
            # sample-tile: 4 matmul chunks -> qubit-1 butterfly (psum->y)
            # -> qubit-0 butterfly (y->out) -> DMA
            with tc.tile_pool(name="pmm", bufs=8,
                              space=bass.MemorySpace.PSUM) as pmm_pool:
                for plane in range(2):
                    base = plane * DIM
                    uts = []
                    for c in range(4):
                        ut = u_pool.tile([128, KT * NW], BF16, tag=f"u{c}")
                        for k in range(KT):
                            nc.sync.dma_start(
                                ut[:, k * NW:(k + 1) * NW],
                                u_d[k, :, base + c * NW:base + (c + 1) * NW])
                        uts.append(ut)
                    for t in range(NTILES):
                        c0_ap = cos_sb[:, t * N:t * N + 1]
                        s0_ap = sin_sb[:, t * N:t * N + 1]
                        c1_ap = cos_sb[:, t * N + 1:t * N + 2]
                        s1_ap = sin_sb[:, t * N + 1:t * N + 2]
                        pms = []
                        for c in range(4):
                            pmm = pmm_pool.tile([128, NW], F32)
                            for k in range(KT):
                                nc.tensor.matmul(
                                    pmm[:],
                                    wts[t][:, k * 128:(k + 1) * 128],
                                    uts[c][:, k * NW:(k + 1) * NW],
                                    start=(k == 0), stop=(k == KT - 1))
                            pms.append(pmm)
                        # qubit-1 butterfly: Bt_c = s1*p_c (ScalarE),
                        # y: pairs (0,1),(2,3) reversed, signs [-,+,+,-]
                        bts = []
                        for c in range(4):
                            bt = tmp_pool.tile([128, NW], F32, tag=f"bt{c}")
                            nc.scalar.mul(bt[:], pms[c][:], s1_ap)
                            bts.append(bt)
                        ys = []
                        for c, (pair, op) in enumerate((
                                (1, SUB), (0, ADD), (3, ADD), (2, SUB))):
                            y = y_pool.tile([128, NW], F32, tag=f"y{c}")
                            nc.vector.scalar_tensor_tensor(
                                y[:], pms[c][:], c1_ap,
                                bts[pair][:, ::-1], MULT, op)
                            ys.append(y)
                        # qubit-0 butterfly: Ct_c = s0*y_c (GpSimd),
                        # out: pairs (0,3),(1,2) reversed, signs [-,-,+,+]
                        cts = []
                        for c in range(4):
                            ct = tmp_pool.tile([128, NW], F32, tag=f"ct{c}")
                            nc.gpsimd.tensor_scalar_mul(ct[:], ys[c][:], s0_ap)
                            cts.append(ct)
                        for c, (pair, op) in enumerate((
                                (3, SUB), (2, SUB), (1, ADD), (0, ADD))):
                            ot = out_pool.tile([128, NW], F32, tag=f"o{c}")
                            nc.vector.scalar_tensor_tensor(
                                ot[:], ys[c][:], c0_ap,
                                cts[pair][:, ::-1], MULT, op)
                            nc.sync.dma_start(
                                out_d[t * 128:(t + 1) * 128,
                                      base + c * NW:base + (c + 1) * NW],
                                ot[:])
    nc.finalize()
    return nc

# ----------------------------------------------------------------- driver

_CACHE = {}


def kernel(X, params):
    X = np.ascontiguousarray(np.asarray(X, dtype=np.float32))
    params = np.asarray(params, dtype=np.float32)

    U = build_u_matrix(params)
    u_bf = np.ascontiguousarray(
        U.reshape(KT, 128, 2 * DIM).astype(ml_dtypes.bfloat16))
    ident = np.eye(128, dtype=ml_dtypes.bfloat16)

    if "nc" not in _CACHE:
        _CACHE["nc"] = build_kernel()
    nc = _CACHE["nc"]

    in_maps = []
    for c in range(NCORES):
        in_maps.append({
            "x": X[c * BSH:(c + 1) * BSH],
            "u": u_bf,
            "ident": ident,
        })
    res = run_bass_kernel_spmd(nc, in_maps, list(range(NCORES)))
    flat = np.concatenate([res.results[c]["out"] for c in range(NCORES)],
                          axis=0)
    out = np.empty((BATCH, DIM, 2), dtype=np.float32)
    out[:, :, 0] = flat[:, 0:DIM]
    out[:, :, 1] = flat[:, DIM:2 * DIM]
    return out


# revision 5
# speedup vs baseline: 3.4370x; 3.4370x over previous
"""Trainium2 Bass kernel for the DataReloadingQNN problem.

Math: layers 0..4 and the shared layer-5 gates B_q = RZ RY RZ are
sample-independent -> shared state v5.  The per-sample part is
    state_b = P . prod_q RY_q(x_bq) . v5          (P = CNOT chain)
RY_q = c_q I + s_q J_q.  Expanding qubits 2..10 gives
    t_b = sum_{m<512} W[b,m] u_m                  (matmul, K=512)
with u_m = J^m v5 and the CNOT permutation P folded into columns.
Qubits 0 and 1 are applied after the matmul as per-sample butterflies;
P is GF(2)-linear, so in P-space the qubit-1 pairing is a ^ 1023 and
the qubit-0 pairing is a ^ 2047.  U columns are stored de-interleaved
(re plane then im plane) and chunks 1,3 of each plane are stored
column-REVERSED, which turns both pairings into same-index chunk swaps:
    q1: chunk c <-> c^1 (within 1024-halves), signs [-,+,+,-]
    q0: chunk c <-> c^3 (across halves),      signs [-,-,+,+]
Signs are folded into constant +-1 tiles (sig1 = [+,-,-,+],
sig0 = [+,+,-,-]), so each butterfly is two wide contiguous
scalar_tensor_tensor ops.  The host un-reverses chunks 1,3 at the end.
Device work per core (1024 samples):
  1. cos/sin of x/2 on ScalarE
  2. W (128 x 512 per sample-tile): tiny doublings + one broadcast
     outer product on VectorE; PE transpose (ScalarE copies out)
  3. t = W @ U as bf16 matmuls, K=512 (PSUM half-tiles of 2 banks)
  4. butterflies on VectorE (bf16 intermediates), DMA out bf16
Inputs are sharded batch-wise across 8 cores; U (params-derived)
replicated.  Host converts bf16 -> f32 and reassembles (B, 2048, 2).
"""
import numpy as np
import ml_dtypes

import concourse.bass as bass
import concourse.bacc as bacc
import concourse.tile as tile
from concourse import mybir
from concourse.bass_utils import run_bass_kernel_spmd

N = 11
DIM = 2048
BATCH = 8192
NCORES = 8
BSH = BATCH // NCORES          # 1024 samples per core
NTILES = BSH // 128            # 8 sample-tiles per core
NPULL = 2                      # qubits 0,1 pulled out of the expansion
K = 512                        # contraction dim (qubits 2..10 expanded)
KT = K // 128                  # 4 k-tiles
NW = 512                       # columns per storage chunk
F32 = mybir.dt.float32
BF16 = mybir.dt.bfloat16

# storage permutation: chunks 1,3 of each plane column-reversed
IDX = np.r_[0:512, np.arange(1023, 511, -1), 1024:1536,
            np.arange(2047, 1535, -1)]

# ---------------------------------------------------------------- host math

def _rz(phi):
    e = np.exp(-0.5j * phi)
    return np.array([[e, 0], [0, np.conj(e)]], dtype=np.complex128)


def _ry(theta):
    t = 0.5 * theta
    c, s = np.cos(t), np.sin(t)
    return np.array([[c, -s], [s, c]], dtype=np.complex128)


def _apply_1q_rows(rows, U, q):
    R = rows.shape[0]
    st = rows.reshape(R, 2 ** q, 2, 2 ** (N - 1 - q))
    st = np.einsum('ab,rxby->rxay', U, st)
    return st.reshape(R, DIM)


def _apply_cnot_rows(rows, c):
    R = rows.shape[0]
    st = rows.reshape(R, 2 ** c, 2, 2, 2 ** (N - 2 - c))
    st = np.stack([st[:, :, 0], st[:, :, 1, ::-1]], axis=2)
    return st.reshape(R, DIM)


def build_u_matrix(params):
    """(6,11,3) f32 -> U (512, 4096) f64 in device storage order."""
    p = params.astype(np.float64)
    v = np.zeros((1, DIM), dtype=np.complex128)
    v[0, 0] = 1.0
    for l in range(5):
        for q in range(N):
            v = _apply_1q_rows(v, _rz(p[l, q, 0]), q)
            v = _apply_1q_rows(v, _ry(p[l, q, 1]), q)
            v = _apply_1q_rows(v, _rz(p[l, q, 2]), q)
        for c in range(N - 1):
            v = _apply_cnot_rows(v, c)
    for q in range(N):
        B = _rz(p[5, q, 2]) @ _ry(p[5, q, 1]) @ _rz(p[5, q, 0])
        v = _apply_1q_rows(v, B, q)

    J = np.array([[0, -1], [1, 0]], dtype=np.complex128)
    rows = v
    for q in range(NPULL, N):      # expand qubits 2..10; bit i <-> qubit i+2
        rc = _apply_1q_rows(rows, J, q)
        rows = np.concatenate([rows, rc], axis=0)

    g = np.arange(DIM)[None, :]
    for c in range(N - 1):
        g = _apply_cnot_rows(g.astype(np.float64), c).astype(np.int64)
    rows = rows[:, g[0]]           # fold CNOT permutation
    rows = rows[:, IDX]            # storage order (chunks 1,3 reversed)

    U = np.empty((K, 2 * DIM), dtype=np.float64)
    U[:, 0:DIM] = rows.real
    U[:, DIM:2 * DIM] = rows.imag
    return U

# ------------------------------------------------------------- bass kernel

def build_kernel():
    nc = bacc.Bacc()
    x_d = nc.dram_tensor("x", (BSH, N), F32, kind="ExternalInput")
    u_d = nc.dram_tensor("u", (KT, 128, 2 * DIM), BF16, kind="ExternalInput")
    id_d = nc.dram_tensor("ident", (128, 128), BF16, kind="ExternalInput")
    out_d = nc.dram_tensor("out", (BSH, 2 * DIM), BF16, kind="ExternalOutput")

    MULT = mybir.AluOpType.mult
    ADD = mybir.AluOpType.add

    NQLO, NQHI = 5, 4            # qubits 2..6 -> low bits, 7..10 -> high
    WLO, WHI = 1 << NQLO, 1 << NQHI

    with tile.TileContext(nc) as tc:
        with (
            tc.tile_pool(name="const", bufs=1) as const_pool,
            tc.tile_pool(name="wbuild", bufs=2) as wbuild_pool,
            tc.tile_pool(name="wt", bufs=1) as wt_pool,
            tc.tile_pool(name="uin", bufs=1) as u_pool,
            tc.tile_pool(name="ys", bufs=2) as y_pool,
            tc.tile_pool(name="outs", bufs=2) as out_pool,
            tc.tile_pool(name="tmps", bufs=2) as tmp_pool,
        ):
            ident = const_pool.tile([128, 128], BF16)
            nc.gpsimd.dma_start(ident[:], id_d[:])

            # x: (1024, 11) -> sbuf (128, 8*11); sample-tile t in cols
            # [t*11, (t+1)*11)
            x_sb = const_pool.tile([128, NTILES * N], F32)
            x_r = x_d.rearrange("(t p) f -> p t f", p=128)
            nc.gpsimd.dma_start(x_sb[:].rearrange("p (t f) -> p t f", f=N), x_r)

            cos_sb = const_pool.tile([128, NTILES * N], F32)
            sin_sb = const_pool.tile([128, NTILES * N], F32)
            hp_t = const_pool.tile([128, 1], F32)
            zr_t = const_pool.tile([128, 1], F32)
            nc.vector.memset(hp_t[:], float(np.pi / 2))
            nc.vector.memset(zr_t[:], 0.0)
            # cos(t) = sin(pi/2 - t): keeps Sin args in (-pi/2, pi/2]
            nc.scalar.activation(cos_sb[:], x_sb[:],
                                 mybir.ActivationFunctionType.Sin,
                                 bias=hp_t[:], scale=-0.5)
            nc.scalar.activation(sin_sb[:], x_sb[:],
                                 mybir.ActivationFunctionType.Sin,
                                 bias=zr_t[:], scale=0.5)

            # sign tiles: sig1 = [+,-,-,+], sig0 = [+,+,-,-] per 512 cols
            sig1 = const_pool.tile([128, 2 * DIM // 2], BF16)  # (128, 2048)
            sig0 = const_pool.tile([128, 2 * DIM // 2], BF16)
            for c, v in enumerate((1.0, -1.0, -1.0, 1.0)):
                nc.vector.memset(sig1[:, c * NW:(c + 1) * NW], v)
            nc.vector.memset(sig0[:, 0:2 * NW], 1.0)
            nc.vector.memset(sig0[:, 2 * NW:4 * NW], -1.0)

            # all of U up front: 8 chunk-tiles (plane, chunk)
            uts = [[None] * 4 for _ in range(2)]
            for plane in range(2):
                for c in range(4):
                    ut = u_pool.tile([128, KT * NW], BF16, tag=f"u{plane}{c}")
                    for k in range(KT):
                        nc.sync.dma_start(
                            ut[:, k * NW:(k + 1) * NW],
                            u_d[k, :, plane * DIM + c * NW:
                                plane * DIM + (c + 1) * NW])
                    uts[plane][c] = ut

            # Phase A: W per sample-tile = outer(wHigh, wLow), bit i of the
            # expansion <-> qubit i+2 (low bits = qubits 2..6)
            wts = []
            with tc.tile_pool(name="ptr", bufs=2,
                              space=bass.MemorySpace.PSUM) as ptr_pool:
                for t in range(NTILES):
                    col = t * N + NPULL
                    wlo = wbuild_pool.tile([128, WLO], F32, tag="wlo")
                    wlob = wbuild_pool.tile([128, WLO], F32, tag="wlob")
                    nc.vector.tensor_copy(wlo[:, 0:1], cos_sb[:, col:col + 1])
                    nc.vector.tensor_copy(wlo[:, 1:2], sin_sb[:, col:col + 1])
                    cur, nxt = wlo, wlob
                    for j in range(1, NQLO):
                        half = 1 << j
                        nc.vector.tensor_scalar_mul(
                            nxt[:, 0:half], cur[:, 0:half],
                            cos_sb[:, col + j:col + j + 1])
                        nc.vector.tensor_scalar_mul(
                            nxt[:, half:2 * half], cur[:, 0:half],
                            sin_sb[:, col + j:col + j + 1])
                        cur, nxt = nxt, cur
                    wlo_f = cur

                    colh = col + NQLO
                    whi = wbuild_pool.tile([128, WHI], F32, tag="whi")
                    whib = wbuild_pool.tile([128, WHI], F32, tag="whib")
                    nc.vector.tensor_copy(whi[:, 0:1], cos_sb[:, colh:colh + 1])
                    nc.vector.tensor_copy(whi[:, 1:2], sin_sb[:, colh:colh + 1])
                    cur, nxt = whi, whib
                    for j in range(1, NQHI):
                        half = 1 << j
                        nc.vector.tensor_scalar_mul(
                            nxt[:, 0:half], cur[:, 0:half],
                            cos_sb[:, colh + j:colh + j + 1])
                        nc.vector.tensor_scalar_mul(
                            nxt[:, half:2 * half], cur[:, 0:half],
                            sin_sb[:, colh + j:colh + j + 1])
                        cur, nxt = nxt, cur
                    whi_f = cur

                    # W[b, i*WLO + j] = wHigh[b,i] * wLow[b,j], bf16
                    wbf = wbuild_pool.tile([128, K], BF16, tag="wbf")
                    av = whi_f[:].rearrange("p (i u) -> p i u", u=1) \
                        .broadcast_to((128, WHI, WLO))
                    bv = wlo_f[:].rearrange("p (u j) -> p u j", u=1) \
                        .broadcast_to((128, WHI, WLO))
                    ov = wbf[:].rearrange("p (i j) -> p i j", j=WLO)
                    nc.vector.tensor_tensor(ov, av, bv, MULT)

                    wt = wt_pool.tile([128, KT * 128], BF16, tag=f"wt{t}")
                    for k in range(KT):
                        ptr = ptr_pool.tile([128, 128], BF16)
                        nc.tensor.transpose(
                            ptr[:], wbf[:, k * 128:(k + 1) * 128], ident[:])
                        nc.scalar.copy(wt[:, k * 128:(k + 1) * 128], ptr[:])
                    wts.append(wt)

            # Phase B: matmuls into (128,1024) psum halves; qubit-1
            # butterfly per half (chunk pairs (0,1),(2,3) same-index);
            # qubit-0 butterfly full-width (pairs (0,3),(1,2))
            with tc.tile_pool(name="pmm", bufs=4,
                              space=bass.MemorySpace.PSUM) as pmm_pool:
                for t in range(NTILES):
                    c0_ap = cos_sb[:, t * N:t * N + 1]
                    s0_ap = sin_sb[:, t * N:t * N + 1]
                    c1_ap = cos_sb[:, t * N + 1:t * N + 2]
                    s1_ap = sin_sb[:, t * N + 1:t * N + 2]
                    for plane in range(2):
                        y = y_pool.tile([128, 2 * DIM // 2], BF16, tag="y")
                        for h in range(2):
                            pm = pmm_pool.tile([128, 2 * NW], F32)
                            for c in range(2):
                                for k in range(KT):
                                    nc.tensor.matmul(
                                        pm[:, c * NW:(c + 1) * NW],
                                        wts[t][:, k * 128:(k + 1) * 128],
                                        uts[plane][2 * h + c][
                                            :, k * NW:(k + 1) * NW],
                                        start=(k == 0), stop=(k == KT - 1))
                            # bt = (pm * s1) * sig1[half]; y_h = pm*c1 + swap(bt)
                            bt = tmp_pool.tile([128, 2 * NW], BF16, tag="bt")
                            nc.vector.scalar_tensor_tensor(
                                bt[:], pm[:], s1_ap,
                                sig1[:, h * 2 * NW:(h + 1) * 2 * NW],
                                MULT, MULT)
                            bt_sw = bt[:].rearrange(
                                "p (c j) -> p c j", j=NW)[:, ::-1, :]
                            yv = y[:, h * 2 * NW:(h + 1) * 2 * NW]
                            nc.vector.scalar_tensor_tensor(
                                yv, pm[:], c1_ap, bt_sw, MULT, ADD)
                        # ct = (y * s0) * sig0; out = y*c0 + swap2(ct)
                        ct = tmp_pool.tile([128, 2 * DIM // 2], BF16, tag="ct")
                        nc.vector.scalar_tensor_tensor(
                            ct[:], y[:], s0_ap, sig0[:], MULT, MULT)
                        ct_sw = ct[:].rearrange(
                            "p (g c j) -> p g c j", g=2, j=NW)[:, ::-1, ::-1, :]
                        ot = out_pool.tile([128, 2 * DIM // 2], BF16, tag="o")
                        nc.vector.scalar_tensor_tensor(
                            ot[:], y[:], c0_ap, ct_sw, MULT, ADD)
                        nc.sync.dma_start(
                            out_d[t * 128:(t + 1) * 128,
                                  plane * DIM:(plane + 1) * DIM],
                            ot[:])
    nc.finalize()
    return nc

# ----------------------------------------------------------------- driver

_CACHE = {}


def make_inputs(X, params):
    X = np.ascontiguousarray(np.asarray(X, dtype=np.float32))
    params = np.asarray(params, dtype=np.float32)
    U = build_u_matrix(params)
    u_bf = np.ascontiguousarray(
        U.reshape(KT, 128, 2 * DIM).astype(ml_dtypes.bfloat16))
    ident = np.eye(128, dtype=ml_dtypes.bfloat16)
    return [{"x": X[c * BSH:(c + 1) * BSH], "u": u_bf, "ident": ident}
            for c in range(NCORES)]


def postprocess(results):
    flat = np.concatenate([results[c]["out"] for c in range(NCORES)],
                          axis=0).astype(np.float32)
    out = np.empty((BATCH, DIM, 2), dtype=np.float32)
    for p in range(2):
        pl = flat[:, p * DIM:(p + 1) * DIM]
        o = out[:, :, p]
        o[:, 0:512] = pl[:, 0:512]
        o[:, 512:1024] = pl[:, 512:1024][:, ::-1]
        o[:, 1024:1536] = pl[:, 1024:1536]
        o[:, 1536:2048] = pl[:, 1536:2048][:, ::-1]
    return out


def kernel(X, params):
    if "nc" not in _CACHE:
        _CACHE["nc"] = build_kernel()
    nc = _CACHE["nc"]
    in_maps = make_inputs(X, params)
    res = run_bass_kernel_spmd(nc, in_maps, list(range(NCORES)))
    return postprocess(res.results)


# revision 8
# speedup vs baseline: 5.5445x; 1.6132x over previous
"""Trainium2 Bass kernel for the DataReloadingQNN problem.

Math: layers 0..4 and the shared layer-5 gates B_q = RZ RY RZ are
sample-independent -> shared state v5.  The per-sample part is
    state_b = P . prod_q RY_q(x_bq) . v5          (P = CNOT chain)
RY_q = c_q I + s_q J_q.  Expanding qubits 2..10 gives
    t_b = sum_{m<512} W[b,m] u_m                  (matmul, K=512)
with u_m = J^m v5 and the CNOT permutation P folded into columns.
Qubits 0 and 1 are applied after the matmul as per-sample butterflies;
P is GF(2)-linear, so in P-space the qubit-1 pairing is a ^ 1023 and
the qubit-0 pairing is a ^ 2047.  U columns are stored de-interleaved
(re plane then im plane) and chunks 1,3 of each plane are stored
column-REVERSED, which turns both pairings into same-index chunk swaps:
    q1: chunk c <-> c^1 (within 1024-halves), signs [-,+,+,-]
    q0: chunk c <-> c^3 (across halves),      signs [-,-,+,+]
Signs are folded into constant +-1 tiles (sig1 = [+,-,-,+],
sig0 = [+,+,-,-]), so each butterfly is two wide contiguous
scalar_tensor_tensor ops.  The host un-reverses chunks 1,3 at the end.
Device work per core (1024 samples):
  1. cos/sin of x/2 on ScalarE
  2. W (128 x 512 per sample-tile): tiny doublings + one broadcast
     outer product on VectorE; PE transpose (ScalarE copies out)
  3. t = W @ U as bf16 matmuls, K=512 (PSUM half-tiles of 2 banks)
  4. butterflies on VectorE (bf16 intermediates), DMA out bf16
Inputs are sharded batch-wise across 8 cores; U (params-derived)
replicated.  Host converts bf16 -> f32 and reassembles (B, 2048, 2).
"""
import numpy as np
import ml_dtypes

import concourse.bass as bass
import concourse.bacc as bacc
import concourse.tile as tile
from concourse import mybir
from concourse.bass_utils import run_bass_kernel_spmd

N = 11
DIM = 2048
BATCH = 8192
NCORES = 8
BSH = BATCH // NCORES          # 1024 samples per core
NTILES = BSH // 128            # 8 sample-tiles per core
NPULL = 2                      # qubits 0,1 pulled out of the expansion
K = 512                        # contraction dim (qubits 2..10 expanded)
KT = K // 128                  # 4 k-tiles
NW = 512                       # columns per storage chunk
F32 = mybir.dt.float32
BF16 = mybir.dt.bfloat16

# storage permutation: chunks 1,3 of each plane column-reversed
IDX = np.r_[0:512, np.arange(1023, 511, -1), 1024:1536,
            np.arange(2047, 1535, -1)]

# ---------------------------------------------------------------- host math

def _rz(phi):
    e = np.exp(-0.5j * phi)
    return np.array([[e, 0], [0, np.conj(e)]], dtype=np.complex128)


def _ry(theta):
    t = 0.5 * theta
    c, s = np.cos(t), np.sin(t)
    return np.array([[c, -s], [s, c]], dtype=np.complex128)


def _apply_1q_rows(rows, U, q):
    R = rows.shape[0]
    st = rows.reshape(R, 2 ** q, 2, 2 ** (N - 1 - q))
    st = np.einsum('ab,rxby->rxay', U, st)
    return st.reshape(R, DIM)


def _apply_cnot_rows(rows, c):
    R = rows.shape[0]
    st = rows.reshape(R, 2 ** c, 2, 2, 2 ** (N - 2 - c))
    st = np.stack([st[:, :, 0], st[:, :, 1, ::-1]], axis=2)
    return st.reshape(R, DIM)


def build_u_matrix(params):
    """(6,11,3) f32 -> U (512, 4096) f64 in device storage order."""
    p = params.astype(np.float64)
    v = np.zeros((1, DIM), dtype=np.complex128)
    v[0, 0] = 1.0
    for l in range(5):
        for q in range(N):
            v = _apply_1q_rows(v, _rz(p[l, q, 0]), q)
            v = _apply_1q_rows(v, _ry(p[l, q, 1]), q)
            v = _apply_1q_rows(v, _rz(p[l, q, 2]), q)
        for c in range(N - 1):
            v = _apply_cnot_rows(v, c)
    for q in range(N):
        B = _rz(p[5, q, 2]) @ _ry(p[5, q, 1]) @ _rz(p[5, q, 0])
        v = _apply_1q_rows(v, B, q)

    J = np.array([[0, -1], [1, 0]], dtype=np.complex128)
    rows = v
    for q in range(NPULL, N):      # expand qubits 2..10; bit i <-> qubit i+2
        rc = _apply_1q_rows(rows, J, q)
        rows = np.concatenate([rows, rc], axis=0)

    g = np.arange(DIM)[None, :]
    for c in range(N - 1):
        g = _apply_cnot_rows(g.astype(np.float64), c).astype(np.int64)
    rows = rows[:, g[0]]           # fold CNOT permutation
    rows = rows[:, IDX]            # storage order (chunks 1,3 reversed)

    U = np.empty((K, 2 * DIM), dtype=np.float64)
    U[:, 0:DIM] = rows.real
    U[:, DIM:2 * DIM] = rows.imag
    return U

# ------------------------------------------------------------- bass kernel

def build_kernel():
    nc = bacc.Bacc()
    x_d = nc.dram_tensor("x", (BSH, N), F32, kind="ExternalInput")
    u_d = nc.dram_tensor("u", (KT, 128, 2 * DIM), BF16, kind="ExternalInput")
    id_d = nc.dram_tensor("ident", (128, 128), BF16, kind="ExternalInput")
    out_d = nc.dram_tensor("out", (BSH, 2 * DIM), BF16, kind="ExternalOutput")

    MULT = mybir.AluOpType.mult
    ADD = mybir.AluOpType.add
    SUB = mybir.AluOpType.subtract

    NQLO, NQHI = 5, 4            # qubits 2..6 -> low bits, 7..10 -> high
    WLO, WHI = 1 << NQLO, 1 << NQHI

    with tile.TileContext(nc) as tc:
        with (
            tc.tile_pool(name="const", bufs=1) as const_pool,
            tc.tile_pool(name="wbuild", bufs=2) as wbuild_pool,
            tc.tile_pool(name="wt", bufs=1) as wt_pool,
            tc.tile_pool(name="uin", bufs=1) as u_pool,
            tc.tile_pool(name="ys", bufs=2) as y_pool,
            tc.tile_pool(name="outs", bufs=2) as out_pool,
            tc.tile_pool(name="tmps", bufs=2) as tmp_pool,
        ):
            ident = const_pool.tile([128, 128], BF16)
            nc.gpsimd.dma_start(ident[:], id_d[:])

            # x: (1024, 11) -> sbuf (128, 8*11); sample-tile t in cols
            # [t*11, (t+1)*11)
            x_sb = const_pool.tile([128, NTILES * N], F32)
            x_r = x_d.rearrange("(t p) f -> p t f", p=128)
            nc.gpsimd.dma_start(x_sb[:].rearrange("p (t f) -> p t f", f=N), x_r)

            cos_sb = const_pool.tile([128, NTILES * N], F32)
            sin_sb = const_pool.tile([128, NTILES * N], F32)
            hp_t = const_pool.tile([128, 1], F32)
            zr_t = const_pool.tile([128, 1], F32)
            nc.vector.memset(hp_t[:], float(np.pi / 2))
            nc.vector.memset(zr_t[:], 0.0)
            # cos(t) = sin(pi/2 - t): keeps Sin args in (-pi/2, pi/2]
            nc.scalar.activation(cos_sb[:], x_sb[:],
                                 mybir.ActivationFunctionType.Sin,
                                 bias=hp_t[:], scale=-0.5)
            nc.scalar.activation(sin_sb[:], x_sb[:],
                                 mybir.ActivationFunctionType.Sin,
                                 bias=zr_t[:], scale=0.5)

            # all of U up front: 8 chunk-tiles (plane, chunk)
            uts = [[None] * 4 for _ in range(2)]
            for plane in range(2):
                for c in range(4):
                    ut = u_pool.tile([128, KT * NW], BF16, tag=f"u{plane}{c}")
                    for k in range(KT):
                        nc.sync.dma_start(
                            ut[:, k * NW:(k + 1) * NW],
                            u_d[k, :, plane * DIM + c * NW:
                                plane * DIM + (c + 1) * NW])
                    uts[plane][c] = ut

            # Phase A: W per sample-tile = outer(wHigh, wLow), bit i of the
            # expansion <-> qubit i+2 (low bits = qubits 2..6)
            wts = []
            with tc.tile_pool(name="ptr", bufs=2,
                              space=bass.MemorySpace.PSUM) as ptr_pool:
                for t in range(NTILES):
                    col = t * N + NPULL
                    wlo = wbuild_pool.tile([128, WLO], F32, tag="wlo")
                    wlob = wbuild_pool.tile([128, WLO], F32, tag="wlob")
                    nc.vector.tensor_copy(wlo[:, 0:1], cos_sb[:, col:col + 1])
                    nc.vector.tensor_copy(wlo[:, 1:2], sin_sb[:, col:col + 1])
                    cur, nxt = wlo, wlob
                    for j in range(1, NQLO):
                        half = 1 << j
                        nc.vector.tensor_scalar_mul(
                            nxt[:, 0:half], cur[:, 0:half],
                            cos_sb[:, col + j:col + j + 1])
                        nc.vector.tensor_scalar_mul(
                            nxt[:, half:2 * half], cur[:, 0:half],
                            sin_sb[:, col + j:col + j + 1])
                        cur, nxt = nxt, cur
                    wlo_f = cur

                    colh = col + NQLO
                    whi = wbuild_pool.tile([128, WHI], F32, tag="whi")
                    whib = wbuild_pool.tile([128, WHI], F32, tag="whib")
                    nc.vector.tensor_copy(whi[:, 0:1], cos_sb[:, colh:colh + 1])
                    nc.vector.tensor_copy(whi[:, 1:2], sin_sb[:, colh:colh + 1])
                    cur, nxt = whi, whib
                    for j in range(1, NQHI):
                        half = 1 << j
                        nc.vector.tensor_scalar_mul(
                            nxt[:, 0:half], cur[:, 0:half],
                            cos_sb[:, colh + j:colh + j + 1])
                        nc.vector.tensor_scalar_mul(
                            nxt[:, half:2 * half], cur[:, 0:half],
                            sin_sb[:, colh + j:colh + j + 1])
                        cur, nxt = nxt, cur
                    whi_f = cur

                    # W[b, i*WLO + j] = wHigh[b,i] * wLow[b,j], bf16
                    wbf = wbuild_pool.tile([128, K], BF16, tag="wbf")
                    av = whi_f[:].rearrange("p (i u) -> p i u", u=1) \
                        .broadcast_to((128, WHI, WLO))
                    bv = wlo_f[:].rearrange("p (u j) -> p u j", u=1) \
                        .broadcast_to((128, WHI, WLO))
                    ov = wbf[:].rearrange("p (i j) -> p i j", j=WLO)
                    nc.vector.tensor_tensor(ov, av, bv, MULT)

                    wt = wt_pool.tile([128, KT * 128], BF16, tag=f"wt{t}")
                    for k in range(KT):
                        ptr = ptr_pool.tile([128, 128], BF16)
                        nc.tensor.transpose(
                            ptr[:], wbf[:, k * 128:(k + 1) * 128], ident[:])
                        nc.scalar.copy(wt[:, k * 128:(k + 1) * 128], ptr[:])
                    wts.append(wt)

            # Phase B per (tile, plane): matmul into one (128,2048) psum
            # tile (4 banks, k-outer so consecutive matmuls share lhsT);
            # ScalarE computes t1 = s1*p, t2 = c1*p straight from PSUM;
            # VectorE does the chunked +- adds and the stage-0 SBUF muls.
            #   q1: y[c]   = t2[c]  +- t1[c^1],  signs [-,+,+,-]
            #   q0: out[c] = t2'[c] +- t1'[c^3], signs [-,-,+,+]
            with tc.tile_pool(name="pmm", bufs=2,
                              space=bass.MemorySpace.PSUM) as pmm_pool:
                for t in range(NTILES):
                    c0_ap = cos_sb[:, t * N:t * N + 1]
                    s0_ap = sin_sb[:, t * N:t * N + 1]
                    c1_ap = cos_sb[:, t * N + 1:t * N + 2]
                    s1_ap = sin_sb[:, t * N + 1:t * N + 2]
                    for plane in range(2):
                        pm = pmm_pool.tile([128, DIM], F32)
                        for k in range(KT):
                            for c in range(4):
                                nc.tensor.matmul(
                                    pm[:, c * NW:(c + 1) * NW],
                                    wts[t][:, k * 128:(k + 1) * 128],
                                    uts[plane][c][:, k * NW:(k + 1) * NW],
                                    start=(k == 0), stop=(k == KT - 1))
                        t1 = tmp_pool.tile([128, DIM], BF16, tag="t1")
                        t2 = tmp_pool.tile([128, DIM], BF16, tag="t2")
                        nc.scalar.mul(t1[:], pm[:], s1_ap)
                        nc.scalar.mul(t2[:], pm[:], c1_ap)
                        y = y_pool.tile([128, DIM], BF16, tag="y")
                        for c, (pair, op) in enumerate(
                                ((1, SUB), (0, ADD), (3, ADD), (2, SUB))):
                            nc.vector.tensor_tensor(
                                y[:, c * NW:(c + 1) * NW],
                                t2[:, c * NW:(c + 1) * NW],
                                t1[:, pair * NW:(pair + 1) * NW], op)
                        t1p = tmp_pool.tile([128, DIM], BF16, tag="t1p")
                        t2p = tmp_pool.tile([128, DIM], BF16, tag="t2p")
                        nc.vector.tensor_scalar_mul(t1p[:], y[:], s0_ap)
                        nc.vector.tensor_scalar_mul(t2p[:], y[:], c0_ap)
                        ot = out_pool.tile([128, DIM], BF16, tag="o")
                        for c, (pair, op) in enumerate(
                                ((3, SUB), (2, SUB), (1, ADD), (0, ADD))):
                            nc.vector.tensor_tensor(
                                ot[:, c * NW:(c + 1) * NW],
                                t2p[:, c * NW:(c + 1) * NW],
                                t1p[:, pair * NW:(pair + 1) * NW], op)
                        nc.sync.dma_start(
                            out_d[t * 128:(t + 1) * 128,
                                  plane * DIM:(plane + 1) * DIM],
                            ot[:])
    nc.finalize()
    return nc

# ----------------------------------------------------------------- driver

_CACHE = {}


def make_inputs(X, params):
    X = np.ascontiguousarray(np.asarray(X, dtype=np.float32))
    params = np.asarray(params, dtype=np.float32)
    U = build_u_matrix(params)
    u_bf = np.ascontiguousarray(
        U.reshape(KT, 128, 2 * DIM).astype(ml_dtypes.bfloat16))
    ident = np.eye(128, dtype=ml_dtypes.bfloat16)
    return [{"x": X[c * BSH:(c + 1) * BSH], "u": u_bf, "ident": ident}
            for c in range(NCORES)]


def postprocess(results):
    flat = np.concatenate([results[c]["out"] for c in range(NCORES)],
                          axis=0).astype(np.float32)
    out = np.empty((BATCH, DIM, 2), dtype=np.float32)
    for p in range(2):
        pl = flat[:, p * DIM:(p + 1) * DIM]
        o = out[:, :, p]
        o[:, 0:512] = pl[:, 0:512]
        o[:, 512:1024] = pl[:, 512:1024][:, ::-1]
        o[:, 1024:1536] = pl[:, 1024:1536]
        o[:, 1536:2048] = pl[:, 1536:2048][:, ::-1]
    return out


def kernel(X, params):
    if "nc" not in _CACHE:
        _CACHE["nc"] = build_kernel()
    nc = _CACHE["nc"]
    in_maps = make_inputs(X, params)
    res = run_bass_kernel_spmd(nc, in_maps, list(range(NCORES)))
    return postprocess(res.results)


# revision 9
# speedup vs baseline: 5.5830x; 1.0069x over previous
"""Trainium2 Bass kernel for the DataReloadingQNN problem.

Math: layers 0..4 and the shared layer-5 gates B_q = RZ RY RZ are
sample-independent -> shared state v5.  The per-sample part is
    state_b = P . prod_q RY_q(x_bq) . v5          (P = CNOT chain)
RY_q = c_q I + s_q J_q.  Expanding qubits 2..10 gives
    t_b = sum_{m<512} W[b,m] u_m                  (matmul, K=512)
with u_m = J^m v5 and the CNOT permutation P folded into columns.
Qubits 0 and 1 are applied after the matmul as per-sample butterflies;
P is GF(2)-linear, so in P-space the qubit-1 pairing is a ^ 1023 and
the qubit-0 pairing is a ^ 2047.  U columns are stored de-interleaved
(re plane then im plane) and chunks 1,3 of each plane are stored
column-REVERSED, which turns both pairings into same-index chunk swaps:
    q1: chunk c <-> c^1 (within 1024-halves), signs [-,+,+,-]
    q0: chunk c <-> c^3 (across halves),      signs [-,-,+,+]
Signs are folded into constant +-1 tiles (sig1 = [+,-,-,+],
sig0 = [+,+,-,-]), so each butterfly is two wide contiguous
scalar_tensor_tensor ops.  The host un-reverses chunks 1,3 at the end.
Device work per core (1024 samples):
  1. cos/sin of x/2 on ScalarE
  2. W (128 x 512 per sample-tile): tiny doublings + one broadcast
     outer product on VectorE; PE transpose (ScalarE copies out)
  3. t = W @ U as bf16 matmuls, K=512 (PSUM half-tiles of 2 banks)
  4. butterflies on VectorE (bf16 intermediates), DMA out bf16
Inputs are sharded batch-wise across 8 cores; U (params-derived)
replicated.  Host converts bf16 -> f32 and reassembles (B, 2048, 2).
"""
import numpy as np
import ml_dtypes

import concourse.bass as bass
import concourse.bacc as bacc
import concourse.tile as tile
from concourse import mybir
from concourse.bass_utils import run_bass_kernel_spmd

N = 11
DIM = 2048
BATCH = 8192
NCORES = 8
BSH = BATCH // NCORES          # 1024 samples per core
NTILES = BSH // 128            # 8 sample-tiles per core
NPULL = 2                      # qubits 0,1 pulled out of the expansion
K = 512                        # contraction dim (qubits 2..10 expanded)
KT = K // 128                  # 4 k-tiles
NW = 512                       # columns per storage chunk
F32 = mybir.dt.float32
BF16 = mybir.dt.bfloat16

# storage permutation: chunks 1,3 of each plane column-reversed
IDX = np.r_[0:512, np.arange(1023, 511, -1), 1024:1536,
            np.arange(2047, 1535, -1)]

# ---------------------------------------------------------------- host math

def _rz(phi):
    e = np.exp(-0.5j * phi)
    return np.array([[e, 0], [0, np.conj(e)]], dtype=np.complex128)


def _ry(theta):
    t = 0.5 * theta
    c, s = np.cos(t), np.sin(t)
    return np.array([[c, -s], [s, c]], dtype=np.complex128)


def _apply_1q_rows(rows, U, q):
    R = rows.shape[0]
    st = rows.reshape(R, 2 ** q, 2, 2 ** (N - 1 - q))
    st = np.einsum('ab,rxby->rxay', U, st)
    return st.reshape(R, DIM)


def _apply_cnot_rows(rows, c):
    R = rows.shape[0]
    st = rows.reshape(R, 2 ** c, 2, 2, 2 ** (N - 2 - c))
    st = np.stack([st[:, :, 0], st[:, :, 1, ::-1]], axis=2)
    return st.reshape(R, DIM)


def build_u_matrix(params):
    """(6,11,3) f32 -> U (512, 4096) f64 in device storage order."""
    p = params.astype(np.float64)
    v = np.zeros((1, DIM), dtype=np.complex128)
    v[0, 0] = 1.0
    for l in range(5):
        for q in range(N):
            v = _apply_1q_rows(v, _rz(p[l, q, 0]), q)
            v = _apply_1q_rows(v, _ry(p[l, q, 1]), q)
            v = _apply_1q_rows(v, _rz(p[l, q, 2]), q)
        for c in range(N - 1):
            v = _apply_cnot_rows(v, c)
    for q in range(N):
        B = _rz(p[5, q, 2]) @ _ry(p[5, q, 1]) @ _rz(p[5, q, 0])
        v = _apply_1q_rows(v, B, q)

    J = np.array([[0, -1], [1, 0]], dtype=np.complex128)
    rows = v
    for q in range(NPULL, N):      # expand qubits 2..10; bit i <-> qubit i+2
        rc = _apply_1q_rows(rows, J, q)
        rows = np.concatenate([rows, rc], axis=0)

    g = np.arange(DIM)[None, :]
    for c in range(N - 1):
        g = _apply_cnot_rows(g.astype(np.float64), c).astype(np.int64)
    rows = rows[:, g[0]]           # fold CNOT permutation
    rows = rows[:, IDX]            # storage order (chunks 1,3 reversed)

    U = np.empty((K, 2 * DIM), dtype=np.float64)
    U[:, 0:DIM] = rows.real
    U[:, DIM:2 * DIM] = rows.imag
    return U

# ------------------------------------------------------------- bass kernel

def build_kernel():
    nc = bacc.Bacc()
    x_d = nc.dram_tensor("x", (BSH, N), F32, kind="ExternalInput")
    u_d = nc.dram_tensor("u", (KT, 128, 2 * DIM), BF16, kind="ExternalInput")
    id_d = nc.dram_tensor("ident", (128, 128), BF16, kind="ExternalInput")
    out_d = nc.dram_tensor("out", (BSH, 2 * DIM), BF16, kind="ExternalOutput")

    MULT = mybir.AluOpType.mult
    ADD = mybir.AluOpType.add
    SUB = mybir.AluOpType.subtract

    NQLO, NQHI = 5, 4            # qubits 2..6 -> low bits, 7..10 -> high
    WLO, WHI = 1 << NQLO, 1 << NQHI

    with tile.TileContext(nc) as tc:
        with (
            tc.tile_pool(name="const", bufs=1) as const_pool,
            tc.tile_pool(name="wbuild", bufs=2) as wbuild_pool,
            tc.tile_pool(name="wt", bufs=1) as wt_pool,
            tc.tile_pool(name="uin", bufs=1) as u_pool,
            tc.tile_pool(name="ys", bufs=3) as y_pool,
            tc.tile_pool(name="outs", bufs=3) as out_pool,
            tc.tile_pool(name="tmps", bufs=3) as tmp_pool,
        ):
            ident = const_pool.tile([128, 128], BF16)
            nc.sync.dma_start(ident[:], id_d[:])

            # x: (1024, 11) -> sbuf (128, 8*11); sample-tile t in cols
            # [t*11, (t+1)*11)
            x_sb = const_pool.tile([128, NTILES * N], F32)
            x_r = x_d.rearrange("(t p) f -> p t f", p=128)
            nc.sync.dma_start(x_sb[:].rearrange("p (t f) -> p t f", f=N), x_r)

            cos_sb = const_pool.tile([128, NTILES * N], F32)
            sin_sb = const_pool.tile([128, NTILES * N], F32)
            hp_t = const_pool.tile([128, 1], F32)
            zr_t = const_pool.tile([128, 1], F32)
            nc.vector.memset(hp_t[:], float(np.pi / 2))
            nc.vector.memset(zr_t[:], 0.0)
            # cos(t) = sin(pi/2 - t): keeps Sin args in (-pi/2, pi/2]
            nc.scalar.activation(cos_sb[:], x_sb[:],
                                 mybir.ActivationFunctionType.Sin,
                                 bias=hp_t[:], scale=-0.5)
            nc.scalar.activation(sin_sb[:], x_sb[:],
                                 mybir.ActivationFunctionType.Sin,
                                 bias=zr_t[:], scale=0.5)

            # all of U up front: one 3D DMA per (plane, chunk), spread
            # over the two HWDGE queues (sync, scalar), plane 0 first
            uts = [[None] * 4 for _ in range(2)]
            for plane in range(2):
                for c in range(4):
                    ut = u_pool.tile([128, KT * NW], BF16, tag=f"u{plane}{c}")
                    eng = nc.sync if c % 2 == 0 else nc.scalar
                    eng.dma_start(
                        ut[:].rearrange("p (k j) -> p k j", j=NW),
                        u_d[:, :, plane * DIM + c * NW:plane * DIM +
                            (c + 1) * NW].rearrange("k p j -> p k j"))
                    uts[plane][c] = ut

            # Phase A: W per sample-tile = outer(wHigh, wLow), bit i of the
            # expansion <-> qubit i+2 (low bits = qubits 2..6)
            wts = []
            with tc.tile_pool(name="ptr", bufs=2,
                              space=bass.MemorySpace.PSUM) as ptr_pool:
                for t in range(NTILES):
                    col = t * N + NPULL
                    wlo = wbuild_pool.tile([128, WLO], F32, tag="wlo")
                    wlob = wbuild_pool.tile([128, WLO], F32, tag="wlob")
                    nc.vector.tensor_copy(wlo[:, 0:1], cos_sb[:, col:col + 1])
                    nc.vector.tensor_copy(wlo[:, 1:2], sin_sb[:, col:col + 1])
                    cur, nxt = wlo, wlob
                    for j in range(1, NQLO):
                        half = 1 << j
                        nc.vector.tensor_scalar_mul(
                            nxt[:, 0:half], cur[:, 0:half],
                            cos_sb[:, col + j:col + j + 1])
                        nc.vector.tensor_scalar_mul(
                            nxt[:, half:2 * half], cur[:, 0:half],
                            sin_sb[:, col + j:col + j + 1])
                        cur, nxt = nxt, cur
                    wlo_f = cur

                    colh = col + NQLO
                    whi = wbuild_pool.tile([128, WHI], F32, tag="whi")
                    whib = wbuild_pool.tile([128, WHI], F32, tag="whib")
                    nc.vector.tensor_copy(whi[:, 0:1], cos_sb[:, colh:colh + 1])
                    nc.vector.tensor_copy(whi[:, 1:2], sin_sb[:, colh:colh + 1])
                    cur, nxt = whi, whib
                    for j in range(1, NQHI):
                        half = 1 << j
                        nc.vector.tensor_scalar_mul(
                            nxt[:, 0:half], cur[:, 0:half],
                            cos_sb[:, colh + j:colh + j + 1])
                        nc.vector.tensor_scalar_mul(
                            nxt[:, half:2 * half], cur[:, 0:half],
                            sin_sb[:, colh + j:colh + j + 1])
                        cur, nxt = nxt, cur
                    whi_f = cur

                    # W[b, i*WLO + j] = wHigh[b,i] * wLow[b,j], bf16
                    wbf = wbuild_pool.tile([128, K], BF16, tag="wbf")
                    av = whi_f[:].rearrange("p (i u) -> p i u", u=1) \
                        .broadcast_to((128, WHI, WLO))
                    bv = wlo_f[:].rearrange("p (u j) -> p u j", u=1) \
                        .broadcast_to((128, WHI, WLO))
                    ov = wbf[:].rearrange("p (i j) -> p i j", j=WLO)
                    nc.vector.tensor_tensor(ov, av, bv, MULT)

                    wt = wt_pool.tile([128, KT * 128], BF16, tag=f"wt{t}")
                    for k in range(KT):
                        ptr = ptr_pool.tile([128, 128], BF16)
                        nc.tensor.transpose(
                            ptr[:], wbf[:, k * 128:(k + 1) * 128], ident[:])
                        nc.scalar.copy(wt[:, k * 128:(k + 1) * 128], ptr[:])
                    wts.append(wt)

            # Phase B per (tile, plane): matmul into one (128,2048) psum
            # tile (4 banks, k-outer so consecutive matmuls share lhsT);
            # ScalarE computes t1 = s1*p, t2 = c1*p straight from PSUM;
            # VectorE does the chunked +- adds and the stage-0 SBUF muls.
            #   q1: y[c]   = t2[c]  +- t1[c^1],  signs [-,+,+,-]
            #   q0: out[c] = t2'[c] +- t1'[c^3], signs [-,-,+,+]
            with tc.tile_pool(name="pmm", bufs=2,
                              space=bass.MemorySpace.PSUM) as pmm_pool:
                for t in range(NTILES):
                    c0_ap = cos_sb[:, t * N:t * N + 1]
                    s0_ap = sin_sb[:, t * N:t * N + 1]
                    c1_ap = cos_sb[:, t * N + 1:t * N + 2]
                    s1_ap = sin_sb[:, t * N + 1:t * N + 2]
                    for plane in range(2):
                        pm = pmm_pool.tile([128, DIM], F32)
                        for k in range(KT):
                            for c in range(4):
                                nc.tensor.matmul(
                                    pm[:, c * NW:(c + 1) * NW],
                                    wts[t][:, k * 128:(k + 1) * 128],
                                    uts[plane][c][:, k * NW:(k + 1) * NW],
                                    start=(k == 0), stop=(k == KT - 1))
                        t1 = tmp_pool.tile([128, DIM], BF16, tag="t1")
                        t2 = tmp_pool.tile([128, DIM], BF16, tag="t2")
                        nc.scalar.mul(t1[:], pm[:], s1_ap)
                        nc.scalar.mul(t2[:], pm[:], c1_ap)
                        y = y_pool.tile([128, DIM], BF16, tag="y")
                        for c, (pair, op) in enumerate(
                                ((1, SUB), (0, ADD), (3, ADD), (2, SUB))):
                            nc.vector.tensor_tensor(
                                y[:, c * NW:(c + 1) * NW],
                                t2[:, c * NW:(c + 1) * NW],
                                t1[:, pair * NW:(pair + 1) * NW], op)
                        t1p = tmp_pool.tile([128, DIM], BF16, tag="t1p")
                        t2p = tmp_pool.tile([128, DIM], BF16, tag="t2p")
                        nc.vector.tensor_scalar_mul(t1p[:], y[:], s0_ap)
                        nc.vector.tensor_scalar_mul(t2p[:], y[:], c0_ap)
                        ot = out_pool.tile([128, DIM], BF16, tag="o")
                        for c, (pair, op) in enumerate(
                                ((3, SUB), (2, SUB), (1, ADD), (0, ADD))):
                            nc.vector.tensor_tensor(
                                ot[:, c * NW:(c + 1) * NW],
                                t2p[:, c * NW:(c + 1) * NW],
                                t1p[:, pair * NW:(pair + 1) * NW], op)
                        nc.sync.dma_start(
                            out_d[t * 128:(t + 1) * 128,
                                  plane * DIM:(plane + 1) * DIM],
                            ot[:])
    nc.finalize()
    return nc

# ----------------------------------------------------------------- driver

_CACHE = {}


def make_inputs(X, params):
    X = np.ascontiguousarray(np.asarray(X, dtype=np.float32))
    params = np.asarray(params, dtype=np.float32)
    U = build_u_matrix(params)
    u_bf = np.ascontiguousarray(
        U.reshape(KT, 128, 2 * DIM).astype(ml_dtypes.bfloat16))
    ident = np.eye(128, dtype=ml_dtypes.bfloat16)
    return [{"x": X[c * BSH:(c + 1) * BSH], "u": u_bf, "ident": ident}
            for c in range(NCORES)]


def postprocess(results):
    flat = np.concatenate([results[c]["out"] for c in range(NCORES)],
                          axis=0).astype(np.float32)
    out = np.empty((BATCH, DIM, 2), dtype=np.float32)
    for p in range(2):
        pl = flat[:, p * DIM:(p + 1) * DIM]
        o = out[:, :, p]
        o[:, 0:512] = pl[:, 0:512]
        o[:, 512:1024] = pl[:, 512:1024][:, ::-1]
        o[:, 1024:1536] = pl[:, 1024:1536]
        o[:, 1536:2048] = pl[:, 1536:2048][:, ::-1]
    return out


def kernel(X, params):
    if "nc" not in _CACHE:
        _CACHE["nc"] = build_kernel()
    nc = _CACHE["nc"]
    in_maps = make_inputs(X, params)
    res = run_bass_kernel_spmd(nc, in_maps, list(range(NCORES)))
    return postprocess(res.results)


# revision 10
# speedup vs baseline: 5.8458x; 1.0471x over previous
"""Trainium2 Bass kernel for the DataReloadingQNN problem.

Math: layers 0..4 and the shared layer-5 gates B_q = RZ RY RZ are
sample-independent -> shared state v5.  The per-sample part is
    state_b = P . prod_q RY_q(x_bq) . v5          (P = CNOT chain)
RY_q = c_q I + s_q J_q.  Expanding qubits 2..10 gives
    t_b = sum_{m<512} W[b,m] u_m                  (matmul, K=512)
with u_m = J^m v5 and the CNOT permutation P folded into columns.
Qubits 0 and 1 are applied after the matmul as per-sample butterflies;
P is GF(2)-linear, so in P-space the qubit-1 pairing is a ^ 1023 and
the qubit-0 pairing is a ^ 2047.  U columns are stored de-interleaved
(re plane then im plane) and chunks 1,3 of each plane are stored
column-REVERSED, which turns both pairings into same-index chunk swaps:
    q1: chunk c <-> c^1 (within 1024-halves), signs [-,+,+,-]
    q0: chunk c <-> c^3 (across halves),      signs [-,-,+,+]
Signs are folded into constant +-1 tiles (sig1 = [+,-,-,+],
sig0 = [+,+,-,-]), so each butterfly is two wide contiguous
scalar_tensor_tensor ops.  The host un-reverses chunks 1,3 at the end.
Device work per core (1024 samples):
  1. cos/sin of x/2 on ScalarE
  2. W (128 x 512 per sample-tile): tiny doublings + one broadcast
     outer product on VectorE; PE transpose (ScalarE copies out)
  3. t = W @ U as bf16 matmuls, K=512 (PSUM half-tiles of 2 banks)
  4. butterflies on VectorE (bf16 intermediates), DMA out bf16
Inputs are sharded batch-wise across 8 cores; U (params-derived)
replicated.  Host converts bf16 -> f32 and reassembles (B, 2048, 2).
"""
import numpy as np
import ml_dtypes

import concourse.bass as bass
import concourse.bacc as bacc
import concourse.tile as tile
from concourse import mybir
from concourse.bass_utils import run_bass_kernel_spmd

N = 11
DIM = 2048
BATCH = 8192
NCORES = 8
BSH = BATCH // NCORES          # 1024 samples per core
NTILES = BSH // 128            # 8 sample-tiles per core
NPULL = 2                      # qubits 0,1 pulled out of the expansion
K = 512                        # contraction dim (qubits 2..10 expanded)
KT = K // 128                  # 4 k-tiles
NW = 512                       # columns per storage chunk
F32 = mybir.dt.float32
BF16 = mybir.dt.bfloat16

# storage permutation: chunks 1,3 of each plane column-reversed
IDX = np.r_[0:512, np.arange(1023, 511, -1), 1024:1536,
            np.arange(2047, 1535, -1)]

# ---------------------------------------------------------------- host math

def _rz(phi):
    e = np.exp(-0.5j * phi)
    return np.array([[e, 0], [0, np.conj(e)]], dtype=np.complex128)


def _ry(theta):
    t = 0.5 * theta
    c, s = np.cos(t), np.sin(t)
    return np.array([[c, -s], [s, c]], dtype=np.complex128)


def _apply_1q_rows(rows, U, q):
    R = rows.shape[0]
    st = rows.reshape(R, 2 ** q, 2, 2 ** (N - 1 - q))
    st = np.einsum('ab,rxby->rxay', U, st)
    return st.reshape(R, DIM)


def _apply_cnot_rows(rows, c):
    R = rows.shape[0]
    st = rows.reshape(R, 2 ** c, 2, 2, 2 ** (N - 2 - c))
    st = np.stack([st[:, :, 0], st[:, :, 1, ::-1]], axis=2)
    return st.reshape(R, DIM)


def build_u_matrix(params):
    """(6,11,3) f32 -> U (512, 4096) f64 in device storage order."""
    p = params.astype(np.float64)
    v = np.zeros((1, DIM), dtype=np.complex128)
    v[0, 0] = 1.0
    for l in range(5):
        for q in range(N):
            v = _apply_1q_rows(v, _rz(p[l, q, 0]), q)
            v = _apply_1q_rows(v, _ry(p[l, q, 1]), q)
            v = _apply_1q_rows(v, _rz(p[l, q, 2]), q)
        for c in range(N - 1):
            v = _apply_cnot_rows(v, c)
    for q in range(N):
        B = _rz(p[5, q, 2]) @ _ry(p[5, q, 1]) @ _rz(p[5, q, 0])
        v = _apply_1q_rows(v, B, q)

    J = np.array([[0, -1], [1, 0]], dtype=np.complex128)
    rows = v
    for q in range(NPULL, N):      # expand qubits 2..10; bit i <-> qubit i+2
        rc = _apply_1q_rows(rows, J, q)
        rows = np.concatenate([rows, rc], axis=0)

    g = np.arange(DIM)[None, :]
    for c in range(N - 1):
        g = _apply_cnot_rows(g.astype(np.float64), c).astype(np.int64)
    rows = rows[:, g[0]]           # fold CNOT permutation
    rows = rows[:, IDX]            # storage order (chunks 1,3 reversed)

    U = np.empty((K, 2 * DIM), dtype=np.float64)
    U[:, 0:DIM] = rows.real
    U[:, DIM:2 * DIM] = rows.imag
    return U

# ------------------------------------------------------------- bass kernel

def build_kernel():
    nc = bacc.Bacc()
    x_d = nc.dram_tensor("x", (BSH, N), F32, kind="ExternalInput")
    u_d = nc.dram_tensor("u", (KT, 128, 2 * DIM), BF16, kind="ExternalInput")
    id_d = nc.dram_tensor("ident", (128, 128), BF16, kind="ExternalInput")
    out_d = nc.dram_tensor("out", (BSH, 2 * DIM), BF16, kind="ExternalOutput")

    MULT = mybir.AluOpType.mult
    ADD = mybir.AluOpType.add
    SUB = mybir.AluOpType.subtract

    NQLO, NQHI = 5, 4            # qubits 2..6 -> low bits, 7..10 -> high
    WLO, WHI = 1 << NQLO, 1 << NQHI

    with tile.TileContext(nc) as tc:
        with (
            tc.tile_pool(name="const", bufs=1) as const_pool,
            tc.tile_pool(name="wbuild", bufs=2) as wbuild_pool,
            tc.tile_pool(name="wt", bufs=1) as wt_pool,
            tc.tile_pool(name="uin", bufs=1) as u_pool,
            tc.tile_pool(name="ys", bufs=3) as y_pool,
            tc.tile_pool(name="outs", bufs=3) as out_pool,
            tc.tile_pool(name="tmps", bufs=3) as tmp_pool,
        ):
            ident = const_pool.tile([128, 128], BF16)
            nc.sync.dma_start(ident[:], id_d[:])

            # x: (1024, 11) -> sbuf (128, 8*11); sample-tile t in cols
            # [t*11, (t+1)*11)
            x_sb = const_pool.tile([128, NTILES * N], F32)
            x_r = x_d.rearrange("(t p) f -> p t f", p=128)
            nc.sync.dma_start(x_sb[:].rearrange("p (t f) -> p t f", f=N), x_r)

            cos_sb = const_pool.tile([128, NTILES * N], F32)
            sin_sb = const_pool.tile([128, NTILES * N], F32)
            hp_t = const_pool.tile([128, 1], F32)
            zr_t = const_pool.tile([128, 1], F32)
            nc.vector.memset(hp_t[:], float(np.pi / 2))
            nc.vector.memset(zr_t[:], 0.0)
            # cos(t) = sin(pi/2 - t): keeps Sin args in (-pi/2, pi/2]
            nc.scalar.activation(cos_sb[:], x_sb[:],
                                 mybir.ActivationFunctionType.Sin,
                                 bias=hp_t[:], scale=-0.5)
            nc.scalar.activation(sin_sb[:], x_sb[:],
                                 mybir.ActivationFunctionType.Sin,
                                 bias=zr_t[:], scale=0.5)

            # all of U up front: one 3D DMA per (plane, chunk), spread
            # over the two HWDGE queues (sync, scalar), plane 0 first
            uts = [[None] * 4 for _ in range(2)]
            for plane in range(2):
                for c in range(4):
                    ut = u_pool.tile([128, KT * NW], BF16, tag=f"u{plane}{c}")
                    eng = nc.sync if c % 2 == 0 else nc.scalar
                    eng.dma_start(
                        ut[:].rearrange("p (k j) -> p k j", j=NW),
                        u_d[:, :, plane * DIM + c * NW:plane * DIM +
                            (c + 1) * NW].rearrange("k p j -> p k j"))
                    uts[plane][c] = ut

            # cos/sin pairs interleaved: cs[:, 2*(t*N+q)] = cos, +1 = sin
            cs = const_pool.tile([128, NTILES * N * 2], F32)
            csv = cs[:].rearrange("p (q c) -> p q c", c=2)
            nc.vector.tensor_copy(csv[:, :, 0:1],
                                  cos_sb[:].rearrange("p (q u) -> p q u", u=1))
            nc.vector.tensor_copy(csv[:, :, 1:2],
                                  sin_sb[:].rearrange("p (q u) -> p q u", u=1))

            def double_step(nxt, cur, w, off):
                """nxt[:, 0:2w] = [cur*cos_q | cur*sin_q] in one op."""
                ov = nxt[:, 0:2 * w].rearrange("p (c w) -> p c w", w=w)
                i0 = cur[:, 0:w].rearrange("p (u w) -> p u w", u=1) \
                    .broadcast_to((128, 2, w))
                i1 = cs[:, off:off + 2].rearrange("p (c u) -> p c u", u=1) \
                    .broadcast_to((128, 2, w))
                nc.vector.tensor_tensor(ov, i0, i1, MULT)

            # Phase A: W per sample-tile = outer(wHigh, wLow), bit i of the
            # expansion <-> qubit i+2 (low bits = qubits 2..6)
            wts = []
            with tc.tile_pool(name="ptr", bufs=2,
                              space=bass.MemorySpace.PSUM) as ptr_pool:
                for t in range(NTILES):
                    col = t * N + NPULL
                    wlo = wbuild_pool.tile([128, WLO], F32, tag="wlo")
                    wlob = wbuild_pool.tile([128, WLO], F32, tag="wlob")
                    nc.vector.tensor_copy(wlo[:, 0:2], cs[:, 2 * col:2 * col + 2])
                    cur, nxt = wlo, wlob
                    for j in range(1, NQLO):
                        double_step(nxt, cur, 1 << j, 2 * (col + j))
                        cur, nxt = nxt, cur
                    wlo_f = cur

                    colh = col + NQLO
                    whi = wbuild_pool.tile([128, WHI], F32, tag="whi")
                    whib = wbuild_pool.tile([128, WHI], F32, tag="whib")
                    nc.vector.tensor_copy(whi[:, 0:2],
                                          cs[:, 2 * colh:2 * colh + 2])
                    cur, nxt = whi, whib
                    for j in range(1, NQHI):
                        double_step(nxt, cur, 1 << j, 2 * (colh + j))
                        cur, nxt = nxt, cur
                    whi_f = cur

                    # W[b, i*WLO + j] = wHigh[b,i] * wLow[b,j], bf16
                    wbf = wbuild_pool.tile([128, K], BF16, tag="wbf")
                    av = whi_f[:].rearrange("p (i u) -> p i u", u=1) \
                        .broadcast_to((128, WHI, WLO))
                    bv = wlo_f[:].rearrange("p (u j) -> p u j", u=1) \
                        .broadcast_to((128, WHI, WLO))
                    ov = wbf[:].rearrange("p (i j) -> p i j", j=WLO)
                    nc.vector.tensor_tensor(ov, av, bv, MULT)

                    wt = wt_pool.tile([128, KT * 128], BF16, tag=f"wt{t}")
                    for k in range(KT):
                        ptr = ptr_pool.tile([128, 128], BF16)
                        nc.tensor.transpose(
                            ptr[:], wbf[:, k * 128:(k + 1) * 128], ident[:])
                        nc.vector.tensor_copy(wt[:, k * 128:(k + 1) * 128], ptr[:])
                    wts.append(wt)

            # Phase B per (tile, plane): matmul into one (128,2048) psum
            # tile (4 banks, k-outer so consecutive matmuls share lhsT);
            # ScalarE computes t1 = s1*p, t2 = c1*p straight from PSUM;
            # VectorE does the chunked +- adds and the stage-0 SBUF muls.
            #   q1: y[c]   = t2[c]  +- t1[c^1],  signs [-,+,+,-]
            #   q0: out[c] = t2'[c] +- t1'[c^3], signs [-,-,+,+]
            with tc.tile_pool(name="pmm", bufs=2,
                              space=bass.MemorySpace.PSUM) as pmm_pool:
                for t in range(NTILES):
                    c0_ap = cos_sb[:, t * N:t * N + 1]
                    s0_ap = sin_sb[:, t * N:t * N + 1]
                    c1_ap = cos_sb[:, t * N + 1:t * N + 2]
                    s1_ap = sin_sb[:, t * N + 1:t * N + 2]
                    for plane in range(2):
                        pm = pmm_pool.tile([128, DIM], F32)
                        for k in range(KT):
                            for c in range(4):
                                nc.tensor.matmul(
                                    pm[:, c * NW:(c + 1) * NW],
                                    wts[t][:, k * 128:(k + 1) * 128],
                                    uts[plane][c][:, k * NW:(k + 1) * NW],
                                    start=(k == 0), stop=(k == KT - 1))
                        t1 = tmp_pool.tile([128, DIM], BF16, tag="t1")
                        t2 = tmp_pool.tile([128, DIM], BF16, tag="t2")
                        nc.scalar.mul(t1[:], pm[:], s1_ap)
                        nc.scalar.mul(t2[:], pm[:], c1_ap)
                        y = y_pool.tile([128, DIM], BF16, tag="y")
                        for c, (pair, op) in enumerate(
                                ((1, SUB), (0, ADD), (3, ADD), (2, SUB))):
                            nc.vector.tensor_tensor(
                                y[:, c * NW:(c + 1) * NW],
                                t2[:, c * NW:(c + 1) * NW],
                                t1[:, pair * NW:(pair + 1) * NW], op)
                        t1p = tmp_pool.tile([128, DIM], BF16, tag="t1p")
                        t2p = tmp_pool.tile([128, DIM], BF16, tag="t2p")
                        nc.vector.tensor_scalar_mul(t1p[:], y[:], s0_ap)
                        nc.vector.tensor_scalar_mul(t2p[:], y[:], c0_ap)
                        ot = out_pool.tile([128, DIM], BF16, tag="o")
                        for c, (pair, op) in enumerate(
                                ((3, SUB), (2, SUB), (1, ADD), (0, ADD))):
                            nc.vector.tensor_tensor(
                                ot[:, c * NW:(c + 1) * NW],
                                t2p[:, c * NW:(c + 1) * NW],
                                t1p[:, pair * NW:(pair + 1) * NW], op)
                        nc.sync.dma_start(
                            out_d[t * 128:(t + 1) * 128,
                                  plane * DIM:(plane + 1) * DIM],
                            ot[:])
    nc.finalize()
    return nc

# ----------------------------------------------------------------- driver

_CACHE = {}


def make_inputs(X, params):
    X = np.ascontiguousarray(np.asarray(X, dtype=np.float32))
    params = np.asarray(params, dtype=np.float32)
    U = build_u_matrix(params)
    u_bf = np.ascontiguousarray(
        U.reshape(KT, 128, 2 * DIM).astype(ml_dtypes.bfloat16))
    ident = np.eye(128, dtype=ml_dtypes.bfloat16)
    return [{"x": X[c * BSH:(c + 1) * BSH], "u": u_bf, "ident": ident}
            for c in range(NCORES)]


def postprocess(results):
    flat = np.concatenate([results[c]["out"] for c in range(NCORES)],
                          axis=0).astype(np.float32)
    out = np.empty((BATCH, DIM, 2), dtype=np.float32)
    for p in range(2):
        pl = flat[:, p * DIM:(p + 1) * DIM]
        o = out[:, :, p]
        o[:, 0:512] = pl[:, 0:512]
        o[:, 512:1024] = pl[:, 512:1024][:, ::-1]
        o[:, 1024:1536] = pl[:, 1024:1536]
        o[:, 1536:2048] = pl[:, 1536:2048][:, ::-1]
    return out


def kernel(X, params):
    if "nc" not in _CACHE:
        _CACHE["nc"] = build_kernel()
    nc = _CACHE["nc"]
    in_maps = make_inputs(X, params)
    res = run_bass_kernel_spmd(nc, in_maps, list(range(NCORES)))
    return postprocess(res.results)
